# revision 19
# baseline (speedup 1.0000x reference)
"""Trainium2 Bass kernel for nn_Conv2d_NN (retrieval_knn).

Reference computation (per batch b):
  xf = x.reshape(B, C, T)                       # T = H*W = 4096, C = 32
  xn = xf / ||xf||_2(channel axis)              # cosine-normalize tokens
  sim = clip(xn^T xn, -1, 1)                    # [T, T]
  vals, idx = top_k(sim, 9)                     # per row, sorted desc
  prime[c,t,k] = vals[t,k] * xf[c, idx[t,k]]
  out[o,t] = sum_{c,k} prime[c,t,k] * w[o,c,k] + bias[o]

Sharding: data-parallel over batch, one batch per NeuronCore (8 cores).

Per-core device algorithm (flash-style fused top-k, sim never hits HBM):
  stage 1: per-token inverse norms via PE transposes + ACT square-accum,
           normalized xn [C, T] built in SBUF.
  stage 2: per 128-token row block:
    - 8 fp32 matmuls xn_blk^T @ xn -> PSUM [128, 512] tiles
    - ACT evicts PSUM -> SBUF sim row [128, 4096]
    - gpsimd affine_select masks the diagonal (self) to -2
    - DVE max -> top-8 values; DVE max_index -> their column indices
    - slot 0 = self (val 1.0, idx = row token): top-9 assembled
    - gpsimd ap_gather pulls the 9*128 neighbor feature columns out of the
      raw x [32, 4096] SBUF tile (indices shared across channel partitions)
    - gathered columns scaled by vals (partition-broadcast row), then the
      conv contraction = 9 accumulating [32x32]x[32x128] matmuls + bias.

The gathered matrix G uses column order j = (q*9 + k)*16 + pp where the
token is p = q*16 + pp (q in [0,8), pp in [0,16)) and k is the neighbor
slot.  This is the natural "wrapped" order of ap_gather's index tile, is
affine to build from idx9 [128, 9] with one cheap DMA per 16-partition
replica, and keeps each k-slice an affine matmul access pattern whose
column walk order is exactly token order.
"""

import sys

if "/opt/trn_rl_repo" not in sys.path:
    sys.path.insert(0, "/opt/trn_rl_repo")

import numpy as np

B, C, H, W = 8, 32, 64, 64
T = H * W          # 4096
KNN = 9            # neighbors
NCORES = 8
RBS = 128          # row-block size (tokens per block)
NRB = T // RBS     # 32
CBS = 512          # col-block size (matmul moving dim)
NCB = T // CBS     # 8
O = 32             # conv output channels
NI = RBS * KNN     # 1152 gathered columns per row block

_CACHE = {}


def _build_program(debug_outs=False):
    import concourse.bass as bass
    import concourse.bacc as bacc
    import concourse.mybir as mybir
    from concourse.tile import TileContext
    from concourse.masks import make_identity

    f32 = mybir.dt.float32
    u32 = mybir.dt.uint32
    i16 = mybir.dt.int16

    nc = bacc.Bacc("TRN2", target_bir_lowering=False, debug=False,
                   num_devices=NCORES)

    xb = nc.dram_tensor("xb", [C, T], f32, kind="ExternalInput")
    wf = nc.dram_tensor("wf", [KNN * C, O], f32, kind="ExternalInput")
    bias = nc.dram_tensor("bias", [O, 1], f32, kind="ExternalInput")
    out = nc.dram_tensor("out", [O, T], f32, kind="ExternalOutput")
    if debug_outs:
        xn_d = nc.dram_tensor("xn_d", [C, T], f32, kind="ExternalOutput")
        vals_d = nc.dram_tensor("vals_d", [T, KNN], f32, kind="ExternalOutput")
        idx_d = nc.dram_tensor("idx_d", [T, KNN], u32, kind="ExternalOutput")
        g_d = nc.dram_tensor("g_d", [T // RBS, C, NI], f32,
                             kind="ExternalOutput")

    AF = mybir.ActivationFunctionType
    ALU = mybir.AluOpType

    with TileContext(nc) as tc:
        with (
            tc.tile_pool(name="const", bufs=1) as cpool,
            tc.tile_pool(name="xdata", bufs=1) as xpool,
        ):
            ident128 = cpool.tile([128, 128], f32)
            make_identity(nc, ident128[:])
            ident32 = cpool.tile([32, 32], f32)
            make_identity(nc, ident32[:])
            iotaP = cpool.tile([128, 1], u32)
            nc.gpsimd.iota(iotaP[:], pattern=[[0, 1]], base=0,
                           channel_multiplier=1)
            wf_sb = []
            for k in range(KNN):
                wf_k = cpool.tile([C, O], f32, name=f"wf_k{k}")
                nc.sync.dma_start(out=wf_k[:],
                                  in_=wf.ap()[k * C:(k + 1) * C, :])
                wf_sb.append(wf_k)

            def wk(k):
                # lhsT [32(c), 32(o)] for neighbor slot k
                return wf_sb[k][:]

            bias_sb = cpool.tile([O, 1], f32)
            nc.sync.dma_start(out=bias_sb[:], in_=bias.ap())

            xb_sb = xpool.tile([C, T], f32)
            nc.sync.dma_start(out=xb_sb[:], in_=xb.ap())
            xn_sb = xpool.tile([C, T], f32)

            # ---- stage 1: inverse norms, normalized xn ----
            with (
                tc.tile_pool(name="s1ps", bufs=3, space="PSUM") as s1ps,
                tc.tile_pool(name="s1sb", bufs=3) as s1sb,
            ):
                for blk in range(NRB):
                    cs = slice(blk * RBS, (blk + 1) * RBS)
                    tp = s1ps.tile([RBS, C], f32, tag="tp")
                    nc.tensor.matmul(tp[:], lhsT=xb_sb[:, cs],
                                     rhs=ident32[:], is_transpose=True)
                    xT_blk = s1sb.tile([RBS, C], f32, tag="xT_blk")
                    nc.scalar.activation(xT_blk[:], tp[:], AF.Copy)
                    sq = s1sb.tile([RBS, C], f32, tag="sq")
                    nsq = s1sb.tile([RBS, 1], f32, tag="nsq")
                    nc.scalar.activation(sq[:], xT_blk[:], AF.Square,
                                         accum_out=nsq[:])
                    nrm = s1sb.tile([RBS, 1], f32, tag="nrm")
                    nc.scalar.activation(nrm[:], nsq[:], AF.Sqrt)
                    rinv = s1sb.tile([RBS, 1], f32, tag="rinv")
                    nc.vector.reciprocal(rinv[:], nrm[:])
                    xnT_blk = s1sb.tile([RBS, C], f32, tag="xnT_blk")
                    nc.vector.tensor_scalar_mul(xnT_blk[:], xT_blk[:], rinv[:])
                    tp2 = s1ps.tile([C, RBS], f32, tag="tp2")
                    nc.tensor.matmul(tp2[:], lhsT=xnT_blk[:],
                                     rhs=ident128[:], is_transpose=True)
                    nc.scalar.activation(xn_sb[:, cs], tp2[:], AF.Copy)

            # ---- stage 2: fused sim + top-k + gather + conv ----
            tc.strict_bb_all_engine_barrier()
            with (
                tc.tile_pool(name="simps", bufs=5, space="PSUM") as simps,
                tc.tile_pool(name="vps", bufs=2, space="PSUM") as vps,
                tc.tile_pool(name="ops", bufs=1, space="PSUM") as ops,
                tc.tile_pool(name="row", bufs=2) as rowpool,
                tc.tile_pool(name="small", bufs=3) as spool,
            ):
                for rb in range(NRB):
                    rs = slice(rb * RBS, (rb + 1) * RBS)
                    simrow = rowpool.tile([RBS, T], f32, tag="simrow")
                    for cb in range(NCB):
                        ps = simps.tile([RBS, CBS], f32, tag="ps")
                        nc.tensor.matmul(
                            ps[:], lhsT=xn_sb[:, rs],
                            rhs=xn_sb[:, cb * CBS:(cb + 1) * CBS],
                            start=True, stop=True)
                        nc.scalar.activation(
                            simrow[:, cb * CBS:(cb + 1) * CBS], ps[:], AF.Copy)
                    # mask the diagonal (self-similarity) to -2
                    nc.gpsimd.affine_select(
                        out=simrow[:, rs], in_=simrow[:, rs],
                        pattern=[[-1, RBS]], channel_multiplier=1, base=0,
                        compare_op=ALU.not_equal, fill=-2.0)
                    vals9 = spool.tile([RBS, KNN], f32, tag="vals9")
                    idx9 = spool.tile([RBS, KNN], u32, tag="idx9")
                    nc.gpsimd.memset(vals9[:, 0:1], 1.0)
                    nc.gpsimd.tensor_scalar_add(idx9[:, 0:1], iotaP[:],
                                                rb * RBS)
                    nc.vector.max(out=vals9[:, 1:KNN], in_=simrow[:])
                    nc.vector.max_index(out=idx9[:, 1:KNN],
                                        in_max=vals9[:, 1:KNN],
                                        in_values=simrow[:])
                    # ---- index tile for ap_gather (wrapped layout) ----
                    idx16 = spool.tile([RBS, KNN], i16, tag="idx16")
                    nc.vector.tensor_copy(idx16[:], idx9[:])
                    # wrap: gather column j = (q*9+k)*16 + pp holds token
                    # p = pp*8 + q, so the source partition walk (pp, q) is
                    # exactly sequential over idx16's 128 partitions.
                    idxw = spool.tile([32, NI // 16], i16, tag="idxw")
                    for g in range(2):
                        nc.sync.dma_start(
                            out=idxw[g * 16:(g + 1) * 16, :].rearrange(
                                "pp (q k) -> pp q k", q=8),
                            in_=idx16[:])
                    # ---- vals broadcast row in gather column order ----
                    vT_ps = vps.tile([KNN, RBS], f32, tag="vT_ps")
                    nc.tensor.matmul(vT_ps[:], lhsT=vals9[:],
                                     rhs=ident128[:], is_transpose=True)
                    vT = spool.tile([KNN, RBS], f32, tag="vT")
                    nc.scalar.activation(vT[:], vT_ps[:], AF.Copy)
                    # vrow[0, k*128 + p] = vals9[p, k] (k-major flatten of vT)
                    vrow = spool.tile([1, NI], f32, tag="vrow")
                    nc.sync.dma_start(out=vrow[:], in_=vT[:])
                    valsb = spool.tile([C, NI], f32, tag="valsb")
                    nc.gpsimd.partition_broadcast(valsb[:], vrow[:])
                    # ---- gather + scale + contract ----
                    gg = spool.tile([C, NI], f32, tag="gg")
                    nc.gpsimd.ap_gather(
                        out_ap=gg[:].rearrange("p (n d) -> p n d", d=1),
                        in_ap=xb_sb[:].rearrange("p (n d) -> p n d", d=1),
                        idxs_ap=idxw[:],
                        channels=32, num_elems=T, d=1, num_idxs=NI)
                    # valsb is k-major; view it in gather column order
                    pp_t = spool.tile([C, NI], f32, tag="pp_t")
                    nc.gpsimd.tensor_tensor(
                        out=pp_t[:].rearrange("c (q k pp) -> c q k pp",
                                              q=8, k=KNN),
                        in0=gg[:].rearrange("c (q k pp) -> c q k pp",
                                            q=8, k=KNN),
                        in1=valsb[:].rearrange("c (k pp q) -> c q k pp",
                                               k=KNN, pp=16),
                        op=ALU.mult)
                    out_ps = ops.tile([O, RBS], f32, tag="out_ps")
                    pview = pp_t[:].rearrange("c (q k pp) -> c k pp q",
                                              q=8, k=KNN)
                    for k in range(KNN):
                        nc.tensor.matmul(out_ps[:], lhsT=wk(k),
                                         rhs=pview[:, k],
                                         start=(k == 0), stop=(k == KNN - 1))
                    out_sb = spool.tile([O, RBS], f32, tag="out_sb")
                    nc.vector.tensor_scalar_add(out_sb[:], out_ps[:],
                                                bias_sb[:])
                    nc.sync.dma_start(out=out.ap()[:, rs], in_=out_sb[:])
                    if debug_outs:
                        nc.sync.dma_start(out=vals_d.ap()[rs, :],
                                          in_=vals9[:])
                        nc.sync.dma_start(out=idx_d.ap()[rs, :], in_=idx9[:])
                        nc.sync.dma_start(out=g_d.ap()[rb], in_=gg[:])
                if debug_outs:
                    nc.sync.dma_start(out=xn_d.ap(), in_=xn_sb[:])
    nc.compile()
    return nc


def _get_program():
    if "nc" not in _CACHE:
        _CACHE["nc"] = _build_program()
    return _CACHE["nc"]


def _prep_inputs(x, weight, bias):
    xf = np.ascontiguousarray(np.asarray(x, dtype=np.float32).reshape(B, C, T))
    # wf[(k,c), o] = weight[o, c, k]
    wfm = np.ascontiguousarray(
        np.asarray(weight, dtype=np.float32).transpose(2, 1, 0).reshape(
            KNN * C, O))
    bp = np.ascontiguousarray(np.asarray(bias, dtype=np.float32).reshape(O, 1))
    return [
        {"xb": np.ascontiguousarray(xf[b]), "wf": wfm, "bias": bp}
        for b in range(B)
    ]


def kernel(x, weight, bias):
    from concourse import bass_utils

    nc = _get_program()
    in_maps = _prep_inputs(x, weight, bias)
    res = bass_utils.run_bass_kernel_spmd(nc, in_maps,
                                          core_ids=list(range(NCORES)))
    out = np.stack([res.results[b]["out"] for b in range(B)])
    return np.ascontiguousarray(out.reshape(B, O, H, W).astype(np.float32))
